# revision 8
# baseline (speedup 1.0000x reference)
"""BoW model Trainium2 kernel (nn_BoWModel_4097398800503).

Computes, for B=2048 rows sharded across 8 NeuronCores (256 rows/core):
  bow   = multi-hot(s)            (set semantics: duplicate tokens count once)
  bow_h = lrelu(bow @ W_bow_h + b_bow_h, 0.2)   == sum of unique W rows + bias
  beo_h = lrelu(x @ W_beo_h + b_beo_h, 0.2)
  outputs: bow_e, beo_e (B x 64), bow_p, beo_p (B x 5)

Per-core device strategy:
  - token dedup on DVE via one shifted-window is_equal + reduce_max; duplicate
    slots get index V (-> zero row of the padded table)
  - 50 indirect-DMA row gathers per 128-row block, accumulated on the PE into
    PSUM via identity matmuls (bias folded in with a ones-row matmul)
  - dense path: PE transposes of x, then matmuls; all head matmuls take the
    transposed hiddens as stationary operands; biases folded via ones-row
  - lrelu(z) computed as max(z, 0.2*z) (the toolchain ignores Lrelu's alpha)
"""
import sys

for _p in ("/opt/trn_rl_repo", "/opt/pypackages"):
    if _p not in sys.path:
        sys.path.append(_p)

import numpy as np
import concourse.bass as bass
import concourse.bacc as bacc
import concourse.mybir as mybir
from concourse.tile import TileContext
from concourse.masks import make_identity
from concourse.bass_utils import run_bass_kernel_spmd

F32 = mybir.dt.float32
I32 = mybir.dt.int32

N_CORES = 8
B = 2048
SEQ = 49 + 1  # 50
V = 100000
HID = 256
BEO = 512
BL = B // N_CORES      # 256 rows per core
NBLK = BL // 128       # 2 blocks of 128 rows
P = 128

_CACHE = {}
import os
STATIC_GATHER = bool(os.environ.get("STATIC_GATHER"))
CLAMP_IDX = int(os.environ.get("CLAMP_IDX", "0"))


def _build():
    nc = bacc.Bacc("TRN2", num_devices=N_CORES)

    s32 = nc.dram_tensor("s32", [BL, SEQ], I32, kind="ExternalInput")
    x = nc.dram_tensor("x", [BL, BEO], F32, kind="ExternalInput")
    Wb = nc.dram_tensor("Wb", [V + 64, HID], F32, kind="ExternalInput")
    Wbeo = nc.dram_tensor("Wbeo", [BEO, HID], F32, kind="ExternalInput")
    # b_bow_h as a [1, 256] bias row; b_beo_h as [128, 2] column chunks
    # (and a 0.2-prescaled copy for the lrelu low branch).
    brow_bow_h = nc.dram_tensor("brow_bow_h", [1, HID], F32, kind="ExternalInput")
    bbeo2 = nc.dram_tensor("bbeo2", [P, 2], F32, kind="ExternalInput")
    bbeo2s = nc.dram_tensor("bbeo2s", [P, 2], F32, kind="ExternalInput")
    # head weights, rows pre-folded: [128, 2*N] where chunk c = rows c*128..c*128+127
    Wh = {}
    bh = {}
    for nm, n in (("bow_pred", 5), ("bow_embd", 64), ("beo_pred", 5), ("beo_embd", 64)):
        Wh[nm] = nc.dram_tensor(f"W_{nm}", [P, 2 * n], F32, kind="ExternalInput")
        bh[nm] = nc.dram_tensor(f"b_{nm}", [1, n], F32, kind="ExternalInput")

    out_bow_p = nc.dram_tensor("out_bow_p", [BL, 5], F32, kind="ExternalOutput")
    out_beo_p = nc.dram_tensor("out_beo_p", [BL, 5], F32, kind="ExternalOutput")
    out_bow_e = nc.dram_tensor("out_bow_e", [BL, 64], F32, kind="ExternalOutput")
    out_beo_e = nc.dram_tensor("out_beo_e", [BL, 64], F32, kind="ExternalOutput")

    with TileContext(nc) as tc:
        with (
            tc.tile_pool(name="const", bufs=1) as cpool,
            tc.tile_pool(name="gather", bufs=10) as gpool,
            tc.tile_pool(name="work", bufs=2) as wpool,
            tc.tile_pool(name="small", bufs=4) as spool,
            tc.tile_pool(name="pacc", bufs=2, space="PSUM") as pacc,
            tc.tile_pool(name="psmall", bufs=2, space="PSUM") as psmall,
        ):
            ident = cpool.tile([P, P], F32)
            make_identity(nc, ident[:])
            ones_row = cpool.tile([1, P], F32)
            nc.vector.memset(ones_row[:], 1.0)

            wbeo_t = cpool.tile([P, 4, HID], F32)  # chunk k = Wbeo rows k*128..
            nc.sync.dma_start(
                out=wbeo_t[:], in_=Wbeo.ap().rearrange("(k p) h -> p k h", p=P)
            )
            brow_bow_t = cpool.tile([1, HID], F32)
            nc.sync.dma_start(out=brow_bow_t[:], in_=brow_bow_h.ap())
            bbeo2_t = cpool.tile([P, 2], F32)
            nc.sync.dma_start(out=bbeo2_t[:], in_=bbeo2.ap())
            bbeo2s_t = cpool.tile([P, 2], F32)
            nc.sync.dma_start(out=bbeo2s_t[:], in_=bbeo2s.ap())
            wh_t = {}
            bh_t = {}
            for nm, n in (("bow_pred", 5), ("bow_embd", 64), ("beo_pred", 5), ("beo_embd", 64)):
                wh_t[nm] = cpool.tile([P, 2 * n], F32, tag=f"wh_{nm}", name=f"wh_{nm}")
                nc.sync.dma_start(out=wh_t[nm][:], in_=Wh[nm].ap())
                bh_t[nm] = cpool.tile([1, n], F32, tag=f"bh_{nm}", name=f"bh_{nm}")
                nc.sync.dma_start(out=bh_t[nm][:], in_=bh[nm].ap())

            for blk in range(NBLK):
                rows = slice(blk * P, (blk + 1) * P)

                # ---------- token indices + dedup ----------
                # s_pad[:, 0:SEQ-1] = -1 pad, s_pad[:, SEQ-1:] = tokens
                s_pad = wpool.tile([P, 2 * SEQ - 1], I32, tag="s_pad")
                nc.vector.memset(s_pad[:, 0 : SEQ - 1], -1)
                nc.sync.dma_start(out=s_pad[:, SEQ - 1 :], in_=s32.ap()[rows, :])
                # eq[p, i, d] = (s[p,i] == s[p,i-1-d]),  d = 0..SEQ-2
                eq_t = wpool.tile([P, SEQ, SEQ - 1], I32, tag="eq")
                base = s_pad[:]
                in0 = bass.AP(
                    tensor=base.tensor,
                    offset=base.offset + (SEQ - 1),
                    ap=[base.ap[0], [1, SEQ], [0, SEQ - 1]],
                )
                in1 = bass.AP(
                    tensor=base.tensor,
                    offset=base.offset + (SEQ - 2),
                    ap=[base.ap[0], [1, SEQ], [-1, SEQ - 1]],
                )
                nc.vector.tensor_tensor(
                    out=eq_t[:], in0=in0, in1=in1, op=mybir.AluOpType.is_equal
                )
                dup = spool.tile([P, SEQ], I32, tag="dup")
                nc.vector.reduce_max(
                    dup[:].unsqueeze(-1), eq_t[:], axis=mybir.AxisListType.X
                )
                # idx = dup ? V : s   ==  max(s, dup * V)
                idx_t = wpool.tile([P, SEQ], I32, tag="idx")
                nc.vector.tensor_scalar_mul(dup[:], dup[:], V)
                nc.vector.tensor_tensor(
                    out=idx_t[:],
                    in0=s_pad[:, SEQ - 1 :],
                    in1=dup[:],
                    op=mybir.AluOpType.max,
                )
                if CLAMP_IDX:
                    nc.vector.tensor_scalar_min(idx_t[:], idx_t[:], CLAMP_IDX)

                # ---------- gather + accumulate on PE ----------
                acc = pacc.tile([P, HID], F32, tag="acc")
                nc.tensor.matmul(
                    out=acc[:], lhsT=ones_row[:], rhs=brow_bow_t[:],
                    start=True, stop=False,
                )
                for i in range(SEQ):
                    g = gpool.tile([P, HID], F32, tag="g")
                    if STATIC_GATHER:
                        nc.sync.dma_start(out=g[:], in_=Wb.ap()[i * P : (i + 1) * P, :])
                    else:
                        nc.gpsimd.indirect_dma_start(
                            out=g[:],
                            out_offset=None,
                            in_=Wb.ap(),
                            in_offset=bass.IndirectOffsetOnAxis(
                                ap=idx_t[:, i : i + 1], axis=0
                            ),
                        )
                    nc.tensor.matmul(
                        out=acc[:], lhsT=ident[:], rhs=g[:],
                        start=False, stop=(i == SEQ - 1),
                    )
                # lrelu: bow_h = max(acc, 0.2*acc)
                t2 = wpool.tile([P, HID], F32, tag="t2")
                nc.scalar.mul(out=t2[:], in_=acc[:], mul=0.2)
                bow_h = wpool.tile([P, HID], F32, tag="bow_h")
                nc.vector.tensor_tensor(
                    out=bow_h[:], in0=acc[:], in1=t2[:], op=mybir.AluOpType.max
                )
                # transpose -> bow_hT chunks [128, 128]
                bow_hT = wpool.tile([P, 2, P], F32, tag="bow_hT")
                for c in range(2):
                    pt = psmall.tile([P, P], F32, tag="ptrans")
                    nc.tensor.transpose(
                        out=pt[:], in_=bow_h[:, c * P : (c + 1) * P], identity=ident[:]
                    )
                    nc.vector.tensor_copy(out=bow_hT[:, c, :], in_=pt[:])

                # ---------- dense (beo) path ----------
                x_t = wpool.tile([P, BEO], F32, tag="x")
                nc.sync.dma_start(out=x_t[:], in_=x.ap()[rows, :])
                xT = wpool.tile([P, 4, P], F32, tag="xT")
                for k in range(4):
                    pt = psmall.tile([P, P], F32, tag="ptrans")
                    nc.tensor.transpose(
                        out=pt[:], in_=x_t[:, k * P : (k + 1) * P], identity=ident[:]
                    )
                    nc.vector.tensor_copy(out=xT[:, k, :], in_=pt[:])
                beo_hT = wpool.tile([P, 2, P], F32, tag="beo_hT")
                for m in range(2):
                    pm = psmall.tile([P, P], F32, tag="pbeo")
                    for k in range(4):
                        nc.tensor.matmul(
                            out=pm[:],
                            lhsT=wbeo_t[:, k, m * P : (m + 1) * P],
                            rhs=xT[:, k, :],
                            start=(k == 0),
                            stop=(k == 3),
                        )
                    # z = pm + bias ; beo_hT = max(z, 0.2 z)
                    za = spool.tile([P, P], F32, tag="za")
                    nc.scalar.activation(
                        out=za[:], in_=pm[:],
                        func=mybir.ActivationFunctionType.Identity,
                        bias=bbeo2_t[:, m : m + 1], scale=1.0,
                    )
                    zb = spool.tile([P, P], F32, tag="zb")
                    nc.scalar.activation(
                        out=zb[:], in_=pm[:],
                        func=mybir.ActivationFunctionType.Identity,
                        bias=bbeo2s_t[:, m : m + 1], scale=0.2,
                    )
                    nc.vector.tensor_tensor(
                        out=beo_hT[:, m, :], in0=za[:], in1=zb[:],
                        op=mybir.AluOpType.max,
                    )

                # ---------- heads ----------
                for nm, n, hT, odram in (
                    ("bow_pred", 5, bow_hT, out_bow_p),
                    ("bow_embd", 64, bow_hT, out_bow_e),
                    ("beo_pred", 5, beo_hT, out_beo_p),
                    ("beo_embd", 64, beo_hT, out_beo_e),
                ):
                    ph = psmall.tile([P, n], F32, tag="phead")
                    for c in range(2):
                        nc.tensor.matmul(
                            out=ph[:],
                            lhsT=hT[:, c, :],
                            rhs=wh_t[nm][:, c * n : (c + 1) * n],
                            start=(c == 0),
                            stop=False,
                        )
                    nc.tensor.matmul(
                        out=ph[:], lhsT=ones_row[:], rhs=bh_t[nm][:],
                        start=False, stop=True,
                    )
                    o_sb = spool.tile([P, n], F32, tag=f"o_{nm}")
                    nc.vector.tensor_copy(out=o_sb[:], in_=ph[:])
                    nc.sync.dma_start(out=odram.ap()[rows, :], in_=o_sb[:])

    nc.finalize()
    return nc


def _prepare_in_maps(inputs):
    s = np.asarray(inputs["s"])
    x = np.ascontiguousarray(np.asarray(inputs["x"], dtype=np.float32))
    W_bow_h = np.asarray(inputs["W_bow_h"], dtype=np.float32)
    b_bow_h = np.asarray(inputs["b_bow_h"], dtype=np.float32)
    W_bow_pred = np.asarray(inputs["W_bow_pred"], dtype=np.float32)
    b_bow_pred = np.asarray(inputs["b_bow_pred"], dtype=np.float32)
    W_bow_embd = np.asarray(inputs["W_bow_embd"], dtype=np.float32)
    b_bow_embd = np.asarray(inputs["b_bow_embd"], dtype=np.float32)
    W_beo_h = np.asarray(inputs["W_beo_h"], dtype=np.float32)
    b_beo_h = np.asarray(inputs["b_beo_h"], dtype=np.float32)
    W_beo_pred = np.asarray(inputs["W_beo_pred"], dtype=np.float32)
    b_beo_pred = np.asarray(inputs["b_beo_pred"], dtype=np.float32)
    W_beo_embd = np.asarray(inputs["W_beo_embd"], dtype=np.float32)
    b_beo_embd = np.asarray(inputs["b_beo_embd"], dtype=np.float32)

    s32 = np.ascontiguousarray(s.astype(np.int32))
    Wb = np.concatenate([W_bow_h, np.zeros((64, HID), np.float32)], axis=0)
    Wb = np.ascontiguousarray(Wb)

    def fold(Wm):  # [256, n] -> [128, 2*n] with chunk c = rows c*128..
        n = Wm.shape[1]
        return np.ascontiguousarray(
            Wm.reshape(2, P, n).transpose(1, 0, 2).reshape(P, 2 * n)
        )

    shared = {
        "Wb": Wb,
        "Wbeo": np.ascontiguousarray(W_beo_h),
        "brow_bow_h": np.ascontiguousarray(b_bow_h[None, :]),
        "bbeo2": np.ascontiguousarray(b_beo_h.reshape(2, P).T),
        "bbeo2s": np.ascontiguousarray(0.2 * b_beo_h.reshape(2, P).T),
        "W_bow_pred": fold(W_bow_pred),
        "b_bow_pred": np.ascontiguousarray(b_bow_pred[None, :]),
        "W_bow_embd": fold(W_bow_embd),
        "b_bow_embd": np.ascontiguousarray(b_bow_embd[None, :]),
        "W_beo_pred": fold(W_beo_pred),
        "b_beo_pred": np.ascontiguousarray(b_beo_pred[None, :]),
        "W_beo_embd": fold(W_beo_embd),
        "b_beo_embd": np.ascontiguousarray(b_beo_embd[None, :]),
    }
    in_maps = []
    for c in range(N_CORES):
        r = slice(c * BL, (c + 1) * BL)
        in_maps.append({"s32": s32[r], "x": x[r], **shared})
    return in_maps


def _postprocess(res):
    bow_e = np.concatenate([res.results[c]["out_bow_e"] for c in range(N_CORES)], axis=0)
    beo_e = np.concatenate([res.results[c]["out_beo_e"] for c in range(N_CORES)], axis=0)
    bow_p = np.concatenate([res.results[c]["out_bow_p"] for c in range(N_CORES)], axis=0)
    beo_p = np.concatenate([res.results[c]["out_beo_p"] for c in range(N_CORES)], axis=0)
    return (bow_e, beo_e, bow_p, beo_p)


def kernel(**inputs):
    if "nc" not in _CACHE:
        _CACHE["nc"] = _build()
    nc = _CACHE["nc"]
    in_maps = _prepare_in_maps(inputs)
    res = run_bass_kernel_spmd(nc, in_maps, core_ids=list(range(N_CORES)))
    return _postprocess(res)
